# revision 1
# baseline (speedup 1.0000x reference)
"""Block-sparse (banded) attention kernel for Trainium2, 8 NeuronCores.

Sharding: data-parallel over batch (2) x tensor-parallel over heads
(16 heads -> 4 per core).  Each core computes its 4 heads' Q/K/V
projections, banded block attention (|r-c| <= 15 blocks, per-block
softmax), and a partial output projection; the host sums the 4 partial
outputs per batch element.

Self-contained: hardcodes all shapes; only needs the concourse tree that
the environment already puts on sys.path.
"""

import sys

for _p in ("/opt/trn_rl_repo",):
    if _p not in sys.path:
        sys.path.insert(0, _p)

from contextlib import ExitStack

import numpy as np

import concourse.bacc as bacc
import concourse.tile as tile
from concourse import bass_utils, mybir

F32 = mybir.dt.float32
F32R = mybir.dt.float32r
BF16 = mybir.dt.bfloat16
EXP = mybir.ActivationFunctionType.Exp

B, S, E = 2, 2048, 1024
H, HD, BLK = 16, 64, 64
NB = S // BLK  # 32 blocks
NCORES = 8
HPC = 4  # heads per core
F = HPC * HD  # 256 local features
BAND = 15
SCALE = HD ** -0.5

# per r8-slab (8 query blocks, q=512) column-block ranges, even-extended
T_SLABS = 4
QS = 512  # q extent per slab
LO = []
NP_T = []
for _t in range(T_SLABS):
    lo = max(0, 8 * _t - BAND)
    hi = min(NB - 1, 8 * _t + 7 + BAND)
    if (hi - lo + 1) % 2 == 1:
        if lo > 0:
            lo -= 1
        else:
            hi += 1
    LO.append(lo)
    NP_T.append((hi - lo + 1) // 2)
MAXP = max(NP_T)  # 16 pairs


def build_nc(debug=False):
    nc = bacc.Bacc("TRN2", target_bir_lowering=False, debug=False)

    xq_d = nc.dram_tensor("xqT", [E, S], F32R, kind="ExternalInput")
    xk_d = nc.dram_tensor("xkT", [E, S], F32R, kind="ExternalInput")
    xv_d = nc.dram_tensor("xvT", [E, S], F32R, kind="ExternalInput")
    wq_d = nc.dram_tensor("wqT", [E, F], F32R, kind="ExternalInput")
    wk_d = nc.dram_tensor("wkT", [E, F], F32R, kind="ExternalInput")
    wv_d = nc.dram_tensor("wvT", [E, F], F32R, kind="ExternalInput")
    wo_d = nc.dram_tensor("woT", [F, E], F32R, kind="ExternalInput")
    sel_d = nc.dram_tensor("selc", [128, MAXP * 32], F32R, kind="ExternalInput")
    bds_d = nc.dram_tensor("bdsel", [32, MAXP * 128], F32R, kind="ExternalInput")
    vm_d = nc.dram_tensor("vmask", [32, T_SLABS * QS], F32R, kind="ExternalInput")
    out_d = nc.dram_tensor("out", [S, E], F32, kind="ExternalOutput")
    if debug:
        qT_d = nc.dram_tensor("qT_dbg", [128, 2 * S], F32, kind="ExternalOutput")
        kT_d = nc.dram_tensor("kT_dbg", [128, 2 * S], F32, kind="ExternalOutput")
        vv_d = nc.dram_tensor("vv_dbg", [128, 16 * F], F32, kind="ExternalOutput")
        at_d = nc.dram_tensor("at_dbg", [F, S], F32, kind="ExternalOutput")

    with tile.TileContext(nc) as tc, ExitStack() as ctx, nc.allow_low_precision(
        reason="float32r pipeline; fp32 data format throughout"
    ):
        pers = ctx.enter_context(tc.tile_pool(name="pers", bufs=1))
        qT = pers.tile([128, 2 * S], F32R, tag="qT")
        kT = pers.tile([128, 2 * S], F32R, tag="kT")
        vv = pers.tile([128, 16 * F], F32R, tag="vv")
        wq = pers.tile([128, 8 * F], F32R, tag="wq")
        wk = pers.tile([128, 8 * F], F32R, tag="wk")
        wv = pers.tile([128, 8 * F], F32R, tag="wv")
        wo = pers.tile([64, 4 * E], F32R, tag="wo")
        selb = pers.tile([128, MAXP * 32], BF16, tag="selb")
        bds = pers.tile([32, MAXP * 128], F32R, tag="bds")
        vm = pers.tile([32, T_SLABS * QS], F32R, tag="vm")

        # k-projection weights first: phase 1 is on the critical path
        nc.sync.dma_start(
            wk[:].rearrange("p (c f) -> p c f", c=8),
            wk_d.ap().rearrange("(c p) f -> p c f", p=128),
        )
        # remaining weights/constants arrive via gpsimd (SWDGE) so they don't
        # queue ahead of the phase-1/2 x-tile loads on the sync ring
        nc.gpsimd.dma_start(
            wv[:].rearrange("p (c f) -> p c f", c=8),
            wv_d.ap().rearrange("(c p) f -> p c f", p=128),
        )
        nc.gpsimd.dma_start(
            wq[:].rearrange("p (c f) -> p c f", c=8),
            wq_d.ap().rearrange("(c p) f -> p c f", p=128),
        )
        nc.gpsimd.dma_start(
            wo[:].rearrange("p (c e) -> p c e", c=4),
            wo_d.ap().rearrange("(c p) e -> p c e", p=64),
        )
        nc.gpsimd.dma_start(selb[:], sel_d.ap())  # SWDGE casts f32 -> bf16
        nc.gpsimd.dma_start(bds[:], bds_d.ap())
        nc.gpsimd.dma_start(vm[:], vm_d.ap())

        # ---- phase 1: k projection (kT layout [f, s]) ----
        with tc.tile_pool(name="xk", bufs=2) as xkp, tc.tile_pool(
            name="psK", bufs=1, space="PSUM"
        ) as pskp:
            psK = pskp.tile([128, 4096], F32)
            for e in range(8):
                xt = xkp.tile([128, S], F32R, tag="xk")
                nc.sync.dma_start(xt[:], xk_d.ap()[e * 128 : (e + 1) * 128, :])
                for fold in range(2):
                    for sc in range(4):
                        nc.tensor.matmul(
                            psK[:, (fold * 4 + sc) * 512 : (fold * 4 + sc + 1) * 512],
                            wk[:, e * F + fold * 128 : e * F + fold * 128 + 128],
                            xt[:, sc * 512 : (sc + 1) * 512],
                            start=(e == 0),
                            stop=(e == 7),
                        )
            for fold in range(2):
                for sc in range(4):
                    nc.scalar.copy(
                        kT[:, fold * S + sc * 512 : fold * S + (sc + 1) * 512],
                        psK[:, (fold * 4 + sc) * 512 : (fold * 4 + sc + 1) * 512],
                    )

        # ---- phase 2: v projection (natural layout [s, f]) ----
        with tc.tile_pool(name="xv", bufs=3) as xvp, tc.tile_pool(
            name="psV", bufs=2, space="PSUM"
        ) as psvp:
            for sc in range(4):
                # one PSUM bank per sub-chunk: accumulation groups must not
                # interleave within a bank
                pvs = [
                    psvp.tile([128, 256], F32, name=f"pv{sub}", tag=f"psV{sub}")
                    for sub in range(4)
                ]
                for e in range(8):
                    xt = xvp.tile([128, 512], F32R, tag="xv")
                    nc.sync.dma_start(
                        xt[:],
                        xv_d.ap()[e * 128 : (e + 1) * 128, sc * 512 : (sc + 1) * 512],
                    )
                    for sub in range(4):
                        nc.tensor.matmul(
                            pvs[sub][:],
                            xt[:, sub * 128 : (sub + 1) * 128],
                            wv[:, e * F : (e + 1) * F],
                            start=(e == 0),
                            stop=(e == 7),
                        )
                for sub in range(4):
                    nc.scalar.copy(
                        vv[:, sc * 1024 + sub * 256 : sc * 1024 + (sub + 1) * 256],
                        pvs[sub][:],
                    )

        # ---- phase 3: q projection + attention + output projection ----
        xqp = ctx.enter_context(tc.tile_pool(name="xq", bufs=3))
        psSp = ctx.enter_context(tc.tile_pool(name="psS", bufs=6, space="PSUM"))
        
        flexp = ctx.enter_context(tc.tile_pool(name="flex", bufs=2, space="PSUM"))
        expp = ctx.enter_context(tc.tile_pool(name="expS", bufs=2))
        ptp = ctx.enter_context(tc.tile_pool(name="pt", bufs=4))
        rcpp = ctx.enter_context(tc.tile_pool(name="rcp", bufs=2))
        attp = ctx.enter_context(tc.tile_pool(name="att", bufs=8))
        outp = ctx.enter_context(tc.tile_pool(name="outsb", bufs=2))

        def unitA(h, t):
            npt = NP_T[t]
            lo = LO[t]
            fold = h // 2
            bp = 64 * (h % 2)  # partition base of this head's qT/kT rows
            expS = expp.tile([128, MAXP * QS], BF16, tag="expS")
            accs = psSp.tile([128, 512], F32, name="accs", tag="psS")
            for j in range(npt):
                c0 = lo + 2 * j
                ps = psSp.tile([128, 512], F32, name="ps", tag="psS")
                nc.tensor.matmul(
                    ps[:],
                    kT[bp : bp + 64, fold * S + c0 * 64 : fold * S + c0 * 64 + 128],
                    qT[bp : bp + 64, fold * S + t * QS : fold * S + (t + 1) * QS],
                    start=True,
                    stop=True,
                )
                nc.scalar.activation(
                    expS[:, j * QS : (j + 1) * QS], ps[:], EXP
                )
                nc.tensor.matmul(
                    accs[0:32, :],
                    selb[:, j * 32 : (j + 1) * 32],
                    expS[:, j * QS : (j + 1) * QS],
                    start=(j == 0),
                    stop=(j == npt - 1),
                )
            return expS, accs

        def unitB(h, t, expS, accs, attn_t):
            npt = NP_T[t]
            lo = LO[t]
            acco = psSp.tile([128, 512], F32, name="acco", tag="psS")
            rc = rcpp.tile([32, 512], F32R, tag="rcp")
            rs1 = rcpp.tile([32, 512], F32, tag="rcs1")
            rs2 = rcpp.tile([32, 512], F32, tag="rcs2")
            nc.vector.reciprocal_approx_accurate(rs2[:], accs[0:32, :], rs1[:])
            nc.vector.tensor_mul(rc[:], rs2[:], vm[:, t * QS : (t + 1) * QS])
            for j in range(npt):
                bt = flexp.tile([128, 512], F32, tag="flex")
                nc.tensor.matmul(
                    bt[:],
                    bds[0 : 2 * npt, j * 128 : (j + 1) * 128],
                    rc[0 : 2 * npt, :],
                    start=True,
                    stop=True,
                )
                pt = ptp.tile([128, 512], F32R, tag="pt")
                nc.vector.tensor_mul(pt[:], expS[:, j * QS : (j + 1) * QS], bt[:])
                cp = lo // 2 + j
                nc.tensor.matmul(
                    acco[0:64, :],
                    vv[:, cp * F + h * 64 : cp * F + h * 64 + 64],
                    pt[:],
                    start=(j == 0),
                    stop=(j == npt - 1),
                )
            nc.scalar.copy(attn_t[:, :], acco[0:64, :])

        def outproj(t, atts):
            for sc2 in range(4):
                ob = outp.tile([128, 1024], F32, tag="outsb")
                for eh in range(2):
                    po = flexp.tile([128, 512], F32, tag="flex")
                    for h in range(HPC):
                        nc.tensor.matmul(
                            po[:],
                            atts[h][:, sc2 * 128 : sc2 * 128 + 128],
                            wo[:, h * E + eh * 512 : h * E + eh * 512 + 512],
                            start=(h == 0),
                            stop=(h == HPC - 1),
                        )
                    nc.scalar.copy(ob[:, eh * 512 : (eh + 1) * 512], po[:])
                row = (4 * t + sc2) * 128
                nc.gpsimd.dma_start(out_d.ap()[row : row + 128, :], ob[:])

        def qproj(sc4):
            pqs = [
                psSp.tile([128, 512], F32, name=f"pq{fold}", tag="psS")
                for fold in range(2)
            ]
            for e in range(8):
                xt = xqp.tile([128, 512], F32R, tag="xq")
                nc.sync.dma_start(
                    xt[:],
                    xq_d.ap()[e * 128 : (e + 1) * 128, sc4 * 512 : (sc4 + 1) * 512],
                )
                for fold in range(2):
                    nc.tensor.matmul(
                        pqs[fold][:],
                        wq[:, e * F + fold * 128 : e * F + fold * 128 + 128],
                        xt[:],
                        start=(e == 0),
                        stop=(e == 7),
                    )
            for fold in range(2):
                nc.scalar.copy(
                    qT[:, fold * S + sc4 * 512 : fold * S + (sc4 + 1) * 512],
                    pqs[fold][:],
                )

        units = [(t, h) for t in range(T_SLABS) for h in range(HPC)]
        pending = None
        atts_by_t = {t: [] for t in range(T_SLABS)}
        for t, h in units:
            if h == 0:
                qproj(t)
            stA = unitA(h, t)
            if pending is not None:
                pt_, ph_, pexpS, pacc, pattn = pending
                unitB(ph_, pt_, pexpS, pacc, pattn)
                atts_by_t[pt_].append(pattn)
                if debug:
                    nc.gpsimd.dma_start(
                        at_d.ap()[ph_ * 64 : ph_ * 64 + 64, pt_ * QS : (pt_ + 1) * QS],
                        pattn[:],
                    )
                if len(atts_by_t[pt_]) == HPC:
                    outproj(pt_, atts_by_t[pt_])
            attn_t = attp.tile([64, 512], F32R, tag="att")
            pending = (t, h, stA[0], stA[1], attn_t)
        pt_, ph_, pexpS, pacc, pattn = pending
        unitB(ph_, pt_, pexpS, pacc, pattn)
        atts_by_t[pt_].append(pattn)
        if debug:
            nc.gpsimd.dma_start(
                at_d.ap()[ph_ * 64 : ph_ * 64 + 64, pt_ * QS : (pt_ + 1) * QS],
                pattn[:],
            )
        outproj(pt_, atts_by_t[pt_])

        if debug:
            nc.gpsimd.dma_start(qT_d.ap(), qT[:])
            nc.gpsimd.dma_start(kT_d.ap(), kT[:])
            nc.gpsimd.dma_start(vv_d.ap(), vv[:])

    nc.compile()
    return nc


_NC_CACHE = []


def _get_nc():
    if not _NC_CACHE:
        _NC_CACHE.append(build_nc())
    return _NC_CACHE[0]


def _host_consts():
    selc = np.zeros((128, MAXP * 32), np.float32)
    for k in range(128):
        for j in range(MAXP):
            selc[k, j * 32 + 2 * j + k // 64] = 1.0
    bdsel = np.zeros((32, MAXP * 128), np.float32)
    for j in range(MAXP):
        for p in range(128):
            bdsel[2 * j + p // 64, j * 128 + p] = 1.0
    vmask = np.zeros((32, T_SLABS * QS), np.float32)
    for t in range(T_SLABS):
        for m in range(2 * NP_T[t]):
            c = LO[t] + m
            for qb in range(QS // BLK):
                r = 8 * t + qb
                if abs(r - c) <= BAND:
                    vmask[m, t * QS + qb * 64 : t * QS + (qb + 1) * 64] = 1.0
    return selc, bdsel, vmask


def kernel(query, key, value, Wq, Wk, Wv, Wo):
    query = np.asarray(query, np.float32)
    key = np.asarray(key, np.float32)
    value = np.asarray(value, np.float32)
    Wq = np.asarray(Wq, np.float32)
    Wk = np.asarray(Wk, np.float32)
    Wv = np.asarray(Wv, np.float32)
    Wo = np.asarray(Wo, np.float32)

    nc = _get_nc()
    selc, bdsel, vmask = _host_consts()

    in_maps = []
    for c in range(NCORES):
        b, g = divmod(c, HPC)
        fs = slice(F * g, F * (g + 1))
        in_maps.append(
            {
                "xqT": np.ascontiguousarray(query[b].T),
                "xkT": np.ascontiguousarray(key[b].T),
                "xvT": np.ascontiguousarray(value[b].T),
                "wqT": np.ascontiguousarray((Wq[fs, :] * SCALE).T),
                "wkT": np.ascontiguousarray(Wk[fs, :].T),
                "wvT": np.ascontiguousarray(Wv[fs, :].T),
                "woT": np.ascontiguousarray(Wo[:, fs].T),
                "selc": selc,
                "bdsel": bdsel,
                "vmask": vmask,
            }
        )

    res = bass_utils.run_bass_kernel_spmd(nc, in_maps, core_ids=list(range(NCORES)))
    out = np.zeros((B, S, E), np.float32)
    for c in range(NCORES):
        b = c // HPC
        out[b] += res.results[c]["out"]
    return out



# revision 3
# speedup vs baseline: 1.1870x; 1.1870x over previous
"""Block-sparse (banded) attention kernel for Trainium2, 8 NeuronCores.

Sharding: data-parallel over batch (2) x tensor-parallel over heads
(16 heads -> 4 per core).  Each core computes its 4 heads' Q/K/V
projections, banded block attention (|r-c| <= 15 blocks, per-block
softmax), and a partial output projection; the host sums the 4 partial
outputs per batch element.

Self-contained: hardcodes all shapes; only needs the concourse tree that
the environment already puts on sys.path.
"""

import sys

for _p in ("/opt/trn_rl_repo",):
    if _p not in sys.path:
        sys.path.insert(0, _p)

from contextlib import ExitStack

import numpy as np
import ml_dtypes

BF16NP = ml_dtypes.bfloat16

import concourse.bacc as bacc
import concourse.tile as tile
from concourse import bass_utils, mybir

F32 = mybir.dt.float32
F32R = mybir.dt.float32r
BF16 = mybir.dt.bfloat16
EXP = mybir.ActivationFunctionType.Exp

B, S, E = 2, 2048, 1024
H, HD, BLK = 16, 64, 64
NB = S // BLK  # 32 blocks
NCORES = 8
HPC = 4  # heads per core
F = HPC * HD  # 256 local features
BAND = 15
SCALE = HD ** -0.5

# per r8-slab (8 query blocks, q=512) column-block ranges, even-extended
T_SLABS = 4
QS = 512  # q extent per slab
LO = []
NP_T = []
for _t in range(T_SLABS):
    lo = max(0, 8 * _t - BAND)
    hi = min(NB - 1, 8 * _t + 7 + BAND)
    if (hi - lo + 1) % 2 == 1:
        if lo > 0:
            lo -= 1
        else:
            hi += 1
    LO.append(lo)
    NP_T.append((hi - lo + 1) // 2)
MAXP = max(NP_T)  # 16 pairs


def build_nc(debug=False):
    nc = bacc.Bacc("TRN2", target_bir_lowering=False, debug=False)

    xq_d = nc.dram_tensor("xqT", [E, S], BF16, kind="ExternalInput")
    xk_d = nc.dram_tensor("xkT", [E, S], BF16, kind="ExternalInput")
    xv_d = nc.dram_tensor("xvT", [E, S], BF16, kind="ExternalInput")
    wq_d = nc.dram_tensor("wqT", [E, F], BF16, kind="ExternalInput")
    wk_d = nc.dram_tensor("wkT", [E, F], BF16, kind="ExternalInput")
    wv_d = nc.dram_tensor("wvT", [E, F], BF16, kind="ExternalInput")
    wo_d = nc.dram_tensor("woT", [F, E], BF16, kind="ExternalInput")
    sel_d = nc.dram_tensor("selc", [128, MAXP * 32], BF16, kind="ExternalInput")
    bds_d = nc.dram_tensor("bdsel", [32, MAXP * 128], BF16, kind="ExternalInput")
    vm_d = nc.dram_tensor("vmask", [32, T_SLABS * QS], BF16, kind="ExternalInput")
    out_d = nc.dram_tensor("out", [S, E], F32, kind="ExternalOutput")
    if debug:
        qT_d = nc.dram_tensor("qT_dbg", [128, 2 * S], F32, kind="ExternalOutput")
        kT_d = nc.dram_tensor("kT_dbg", [128, 2 * S], F32, kind="ExternalOutput")
        vv_d = nc.dram_tensor("vv_dbg", [128, 16 * F], F32, kind="ExternalOutput")
        at_d = nc.dram_tensor("at_dbg", [F, S], F32, kind="ExternalOutput")

    with tile.TileContext(nc) as tc, ExitStack() as ctx, nc.allow_low_precision(
        reason="float32r pipeline; fp32 data format throughout"
    ):
        pers = ctx.enter_context(tc.tile_pool(name="pers", bufs=1))
        qT = pers.tile([128, 2 * S], BF16, tag="qT")
        kT = pers.tile([128, 2 * S], BF16, tag="kT")
        vv = pers.tile([128, 16 * F], BF16, tag="vv")
        wq = pers.tile([128, 8 * F], BF16, tag="wq")
        wk = pers.tile([128, 8 * F], BF16, tag="wk")
        wv = pers.tile([128, 8 * F], BF16, tag="wv")
        wo = pers.tile([64, 4 * E], BF16, tag="wo")
        selb = pers.tile([128, MAXP * 32], BF16, tag="selb")
        bds = pers.tile([32, MAXP * 128], BF16, tag="bds")
        vm = pers.tile([32, T_SLABS * QS], BF16, tag="vm")

        # k-projection weights first: phase 1 is on the critical path
        nc.sync.dma_start(
            wk[:].rearrange("p (c f) -> p c f", c=8),
            wk_d.ap().rearrange("(c p) f -> p c f", p=128),
        )
        # remaining weights/constants arrive via gpsimd (SWDGE) so they don't
        # queue ahead of the phase-1/2 x-tile loads on the sync ring
        nc.gpsimd.dma_start(
            wv[:].rearrange("p (c f) -> p c f", c=8),
            wv_d.ap().rearrange("(c p) f -> p c f", p=128),
        )
        nc.gpsimd.dma_start(
            wq[:].rearrange("p (c f) -> p c f", c=8),
            wq_d.ap().rearrange("(c p) f -> p c f", p=128),
        )
        nc.gpsimd.dma_start(
            wo[:].rearrange("p (c e) -> p c e", c=4),
            wo_d.ap().rearrange("(c p) e -> p c e", p=64),
        )
        nc.gpsimd.dma_start(selb[:], sel_d.ap())  # SWDGE casts f32 -> bf16
        nc.gpsimd.dma_start(bds[:], bds_d.ap())
        nc.gpsimd.dma_start(vm[:], vm_d.ap())

        # ---- phase 1: k projection (kT layout [f, s]) ----
        with tc.tile_pool(name="xk", bufs=2) as xkp, tc.tile_pool(
            name="psK", bufs=1, space="PSUM"
        ) as pskp:
            psK = pskp.tile([128, 4096], F32)
            for e in range(8):
                xt = xkp.tile([128, S], BF16, tag="xk")
                nc.sync.dma_start(xt[:], xk_d.ap()[e * 128 : (e + 1) * 128, :])
                for fold in range(2):
                    for sc in range(4):
                        nc.tensor.matmul(
                            psK[:, (fold * 4 + sc) * 512 : (fold * 4 + sc + 1) * 512],
                            wk[:, e * F + fold * 128 : e * F + fold * 128 + 128],
                            xt[:, sc * 512 : (sc + 1) * 512],
                            start=(e == 0),
                            stop=(e == 7),
                        )
            for fold in range(2):
                for sc in range(4):
                    nc.scalar.copy(
                        kT[:, fold * S + sc * 512 : fold * S + (sc + 1) * 512],
                        psK[:, (fold * 4 + sc) * 512 : (fold * 4 + sc + 1) * 512],
                    )

        # ---- phase 2: v projection (natural layout [s, f]) ----
        with tc.tile_pool(name="xv", bufs=3) as xvp, tc.tile_pool(
            name="psV", bufs=2, space="PSUM"
        ) as psvp:
            for sc in range(4):
                # one PSUM bank per sub-chunk: accumulation groups must not
                # interleave within a bank
                pvs = [
                    psvp.tile([128, 256], F32, name=f"pv{sub}", tag=f"psV{sub}")
                    for sub in range(4)
                ]
                for e in range(8):
                    xt = xvp.tile([128, 512], BF16, tag="xv")
                    nc.sync.dma_start(
                        xt[:],
                        xv_d.ap()[e * 128 : (e + 1) * 128, sc * 512 : (sc + 1) * 512],
                    )
                    for sub in range(4):
                        nc.tensor.matmul(
                            pvs[sub][:],
                            xt[:, sub * 128 : (sub + 1) * 128],
                            wv[:, e * F : (e + 1) * F],
                            start=(e == 0),
                            stop=(e == 7),
                        )
                for sub in range(4):
                    nc.scalar.copy(
                        vv[:, sc * 1024 + sub * 256 : sc * 1024 + (sub + 1) * 256],
                        pvs[sub][:],
                    )

        # ---- phase 3: q projection + attention + output projection ----
        xqp = ctx.enter_context(tc.tile_pool(name="xq", bufs=3))
        psSp = ctx.enter_context(tc.tile_pool(name="psS", bufs=6, space="PSUM"))
        
        flexp = ctx.enter_context(tc.tile_pool(name="flex", bufs=2, space="PSUM"))
        expp = ctx.enter_context(tc.tile_pool(name="expS", bufs=2))
        ptp = ctx.enter_context(tc.tile_pool(name="pt", bufs=4))
        rcpp = ctx.enter_context(tc.tile_pool(name="rcp", bufs=2))
        attp = ctx.enter_context(tc.tile_pool(name="att", bufs=8))
        outp = ctx.enter_context(tc.tile_pool(name="outsb", bufs=2))

        def unitA(h, t):
            npt = NP_T[t]
            lo = LO[t]
            fold = h // 2
            bp = 64 * (h % 2)  # partition base of this head's qT/kT rows
            expS = expp.tile([128, MAXP * QS], BF16, tag="expS")
            accs = psSp.tile([128, 512], F32, name="accs", tag="psS")
            for j in range(npt):
                c0 = lo + 2 * j
                ps = psSp.tile([128, 512], F32, name="ps", tag="psS")
                nc.tensor.matmul(
                    ps[:],
                    kT[bp : bp + 64, fold * S + c0 * 64 : fold * S + c0 * 64 + 128],
                    qT[bp : bp + 64, fold * S + t * QS : fold * S + (t + 1) * QS],
                    start=True,
                    stop=True,
                )
                nc.scalar.activation(
                    expS[:, j * QS : (j + 1) * QS], ps[:], EXP
                )
                nc.tensor.matmul(
                    accs[0:32, :],
                    selb[:, j * 32 : (j + 1) * 32],
                    expS[:, j * QS : (j + 1) * QS],
                    start=(j == 0),
                    stop=(j == npt - 1),
                )
            return expS, accs

        def unitB(h, t, expS, accs, attn_t):
            npt = NP_T[t]
            lo = LO[t]
            acco = psSp.tile([128, 512], F32, name="acco", tag="psS")
            rc = rcpp.tile([32, 512], BF16, tag="rcp")
            rs1 = rcpp.tile([32, 512], F32, tag="rcs1")
            rs2 = rcpp.tile([32, 512], F32, tag="rcs2")
            nc.vector.reciprocal_approx_accurate(rs2[:], accs[0:32, :], rs1[:])
            nc.vector.tensor_mul(rc[:], rs2[:], vm[:, t * QS : (t + 1) * QS])
            for j in range(npt):
                bt = flexp.tile([128, 512], F32, tag="flex")
                nc.tensor.matmul(
                    bt[:],
                    bds[0 : 2 * npt, j * 128 : (j + 1) * 128],
                    rc[0 : 2 * npt, :],
                    start=True,
                    stop=True,
                )
                pt = ptp.tile([128, 512], BF16, tag="pt")
                nc.vector.tensor_mul(pt[:], expS[:, j * QS : (j + 1) * QS], bt[:])
                cp = lo // 2 + j
                nc.tensor.matmul(
                    acco[0:64, :],
                    vv[:, cp * F + h * 64 : cp * F + h * 64 + 64],
                    pt[:],
                    start=(j == 0),
                    stop=(j == npt - 1),
                )
            nc.scalar.copy(attn_t[:, :], acco[0:64, :])

        def outproj(t, atts):
            for sc2 in range(4):
                ob = outp.tile([128, 1024], F32, tag="outsb")
                for eh in range(2):
                    po = flexp.tile([128, 512], F32, tag="flex")
                    for h in range(HPC):
                        nc.tensor.matmul(
                            po[:],
                            atts[h][:, sc2 * 128 : sc2 * 128 + 128],
                            wo[:, h * E + eh * 512 : h * E + eh * 512 + 512],
                            start=(h == 0),
                            stop=(h == HPC - 1),
                        )
                    nc.scalar.copy(ob[:, eh * 512 : (eh + 1) * 512], po[:])
                row = (4 * t + sc2) * 128
                nc.gpsimd.dma_start(out_d.ap()[row : row + 128, :], ob[:])

        def qproj(sc4):
            pqs = [
                psSp.tile([128, 512], F32, name=f"pq{fold}", tag="psS")
                for fold in range(2)
            ]
            for e in range(8):
                xt = xqp.tile([128, 512], BF16, tag="xq")
                nc.sync.dma_start(
                    xt[:],
                    xq_d.ap()[e * 128 : (e + 1) * 128, sc4 * 512 : (sc4 + 1) * 512],
                )
                for fold in range(2):
                    nc.tensor.matmul(
                        pqs[fold][:],
                        wq[:, e * F + fold * 128 : e * F + fold * 128 + 128],
                        xt[:],
                        start=(e == 0),
                        stop=(e == 7),
                    )
            for fold in range(2):
                nc.scalar.copy(
                    qT[:, fold * S + sc4 * 512 : fold * S + (sc4 + 1) * 512],
                    pqs[fold][:],
                )

        units = [(t, h) for t in range(T_SLABS) for h in range(HPC)]
        pending = None
        atts_by_t = {t: [] for t in range(T_SLABS)}
        for t, h in units:
            if h == 0:
                qproj(t)
            stA = unitA(h, t)
            if pending is not None:
                pt_, ph_, pexpS, pacc, pattn = pending
                unitB(ph_, pt_, pexpS, pacc, pattn)
                atts_by_t[pt_].append(pattn)
                if debug:
                    nc.gpsimd.dma_start(
                        at_d.ap()[ph_ * 64 : ph_ * 64 + 64, pt_ * QS : (pt_ + 1) * QS],
                        pattn[:],
                    )
                if len(atts_by_t[pt_]) == HPC:
                    outproj(pt_, atts_by_t[pt_])
            attn_t = attp.tile([64, 512], BF16, tag="att")
            pending = (t, h, stA[0], stA[1], attn_t)
        pt_, ph_, pexpS, pacc, pattn = pending
        unitB(ph_, pt_, pexpS, pacc, pattn)
        atts_by_t[pt_].append(pattn)
        if debug:
            nc.gpsimd.dma_start(
                at_d.ap()[ph_ * 64 : ph_ * 64 + 64, pt_ * QS : (pt_ + 1) * QS],
                pattn[:],
            )
        outproj(pt_, atts_by_t[pt_])

        if debug:
            nc.gpsimd.dma_start(qT_d.ap(), qT[:])
            nc.gpsimd.dma_start(kT_d.ap(), kT[:])
            nc.gpsimd.dma_start(vv_d.ap(), vv[:])

    nc.compile()
    return nc


_NC_CACHE = []


def _get_nc():
    if not _NC_CACHE:
        _NC_CACHE.append(build_nc())
    return _NC_CACHE[0]


def _host_consts():
    selc = np.zeros((128, MAXP * 32), np.float32)
    for k in range(128):
        for j in range(MAXP):
            selc[k, j * 32 + 2 * j + k // 64] = 1.0
    bdsel = np.zeros((32, MAXP * 128), np.float32)
    for j in range(MAXP):
        for p in range(128):
            bdsel[2 * j + p // 64, j * 128 + p] = 1.0
    vmask = np.zeros((32, T_SLABS * QS), np.float32)
    for t in range(T_SLABS):
        for m in range(2 * NP_T[t]):
            c = LO[t] + m
            for qb in range(QS // BLK):
                r = 8 * t + qb
                if abs(r - c) <= BAND:
                    vmask[m, t * QS + qb * 64 : t * QS + (qb + 1) * 64] = 1.0
    return selc, bdsel, vmask


def make_in_maps(query, key, value, Wq, Wk, Wv, Wo):
    query = np.asarray(query, np.float32)
    key = np.asarray(key, np.float32)
    value = np.asarray(value, np.float32)
    Wq = np.asarray(Wq, np.float32)
    Wk = np.asarray(Wk, np.float32)
    Wv = np.asarray(Wv, np.float32)
    Wo = np.asarray(Wo, np.float32)

    selc, bdsel, vmask = _host_consts()

    in_maps = []
    for c in range(NCORES):
        b, g = divmod(c, HPC)
        fs = slice(F * g, F * (g + 1))
        in_maps.append(
            {
                "xqT": np.ascontiguousarray(query[b].T).astype(BF16NP),
                "xkT": np.ascontiguousarray(key[b].T).astype(BF16NP),
                "xvT": np.ascontiguousarray(value[b].T).astype(BF16NP),
                "wqT": np.ascontiguousarray((Wq[fs, :] * SCALE).T).astype(BF16NP),
                "wkT": np.ascontiguousarray(Wk[fs, :].T).astype(BF16NP),
                "wvT": np.ascontiguousarray(Wv[fs, :].T).astype(BF16NP),
                "woT": np.ascontiguousarray(Wo[:, fs].T).astype(BF16NP),
                "selc": selc.astype(BF16NP),
                "bdsel": bdsel.astype(BF16NP),
                "vmask": vmask.astype(BF16NP),
            }
        )
    return in_maps


def kernel(query, key, value, Wq, Wk, Wv, Wo):
    nc = _get_nc()
    in_maps = make_in_maps(query, key, value, Wq, Wk, Wv, Wo)
    res = bass_utils.run_bass_kernel_spmd(nc, in_maps, core_ids=list(range(NCORES)))
    out = np.zeros((B, S, E), np.float32)
    for c in range(NCORES):
        b = c // HPC
        out[b] += res.results[c]["out"]
    return out



# revision 4
# speedup vs baseline: 1.3935x; 1.1739x over previous
"""Block-sparse (banded) attention kernel for Trainium2, 8 NeuronCores.

Sharding: data-parallel over batch (2) x tensor-parallel over heads
(16 heads -> 4 per core).  Each core computes its 4 heads' Q/K/V
projections, banded block attention (|r-c| <= 15 blocks, per-block
softmax), and a partial output projection; the host sums the 4 partial
outputs per batch element.

V2 structure: the band mask is folded into the scores matmul via 32
static contract rows (one-hot q-block indicator on the moving side x
-3e4 band-complement table on the stationary side), so masked scores
exp to exactly 0.  Per-block softmax denominators come from ONE matmul
with a block-membership (+eps) stationary whose output is already
broadcast across partitions; reciprocal runs per pair on the vector
engine.  Each pair only processes its valid contiguous q-range.

Self-contained: hardcodes all shapes; only needs the concourse tree that
the environment already puts on sys.path.
"""

import sys

for _p in ("/opt/trn_rl_repo",):
    if _p not in sys.path:
        sys.path.insert(0, _p)

from contextlib import ExitStack

import numpy as np
import ml_dtypes

import concourse.bacc as bacc
import concourse.tile as tile
from concourse import bass_utils, mybir

F32 = mybir.dt.float32
BF16 = mybir.dt.bfloat16
EXP = mybir.ActivationFunctionType.Exp
BF16NP = ml_dtypes.bfloat16

B, S, E = 2, 2048, 1024
H, HD, BLK = 16, 64, 64
NB = S // BLK  # 32 blocks
NCORES = 8
HPC = 4  # heads per core
F = HPC * HD  # 256 local features
BAND = 15
SCALE = HD ** -0.5
BIGNEG = -30000.0  # masked-score bias; exp underflows to exactly 0 in f32
EPS_BG = 1e-20  # background weight in the sum stationary: keeps denom > 0

# per r8-slab (8 query blocks, q=512) column-block ranges, even-extended
T_SLABS = 4
QS = 512  # q extent per slab
LO = []
NP_T = []
for _t in range(T_SLABS):
    lo = max(0, 8 * _t - BAND)
    hi = min(NB - 1, 8 * _t + 7 + BAND)
    if (hi - lo + 1) % 2 == 1:
        if lo > 0:
            lo -= 1
        else:
            hi += 1
    LO.append(lo)
    NP_T.append((hi - lo + 1) // 2)
MAXP = max(NP_T)  # 16 pairs

# per (slab, pair): valid contiguous local q-block range [lb, ub]
#   union of the two blocks' bands: global r in [c0-15, c0+16]
QRANGE = []  # QRANGE[t][j] = (qlo, qhi) in elements within the slab
PAIR_ORDER = []  # full-width pair first (accumulation-group opener)
for _t in range(T_SLABS):
    rng = []
    for _j in range(NP_T[_t]):
        c0 = LO[_t] + 2 * _j
        lb = max(0, c0 - BAND - 8 * _t)
        ub = min(7, c0 + BAND + 1 - 8 * _t)
        assert lb <= ub
        rng.append((lb * BLK, (ub + 1) * BLK))
    QRANGE.append(rng)
    full = [j for j, (a, b) in enumerate(rng) if b - a == QS]
    order = [full[0]] + [j for j in range(NP_T[_t]) if j != full[0]]
    PAIR_ORDER.append(order)


def build_nc():
    nc = bacc.Bacc("TRN2", target_bir_lowering=False, debug=False)

    xq_d = nc.dram_tensor("xqT", [E, S], BF16, kind="ExternalInput")
    xk_d = nc.dram_tensor("xkT", [E, S], BF16, kind="ExternalInput")
    xv_d = nc.dram_tensor("xvT", [E, S], BF16, kind="ExternalInput")
    wq_d = nc.dram_tensor("wqT", [E, F], BF16, kind="ExternalInput")
    wk_d = nc.dram_tensor("wkT", [E, F], BF16, kind="ExternalInput")
    wv_d = nc.dram_tensor("wvT", [E, F], BF16, kind="ExternalInput")
    wo_d = nc.dram_tensor("woT", [F, E], BF16, kind="ExternalInput")
    qhot_d = nc.dram_tensor("qhot", [32, S], BF16, kind="ExternalInput")
    kband_d = nc.dram_tensor("kband", [32, S], BF16, kind="ExternalInput")
    sbc_d = nc.dram_tensor("sbc", [128, 128], BF16, kind="ExternalInput")
    out_d = nc.dram_tensor("out", [S, E], F32, kind="ExternalOutput")

    with tile.TileContext(nc) as tc, ExitStack() as ctx, nc.allow_low_precision(
        reason="bf16 pipeline; fp32 PSUM accumulate throughout"
    ):
        pers = ctx.enter_context(tc.tile_pool(name="pers", bufs=1))
        qT = pers.tile([96, HPC * S], BF16, tag="qT")
        kT = pers.tile([96, HPC * S], BF16, tag="kT")
        vv = pers.tile([128, 16 * F], BF16, tag="vv")
        wq = pers.tile([128, 8 * F], BF16, tag="wq")
        wk = pers.tile([128, 8 * F], BF16, tag="wk")
        wv = pers.tile([128, 8 * F], BF16, tag="wv")
        wo = pers.tile([64, 4 * E], BF16, tag="wo")
        sbc = pers.tile([128, 128], BF16, tag="sbc")

        # k-projection weights first: phase 1 is on the critical path
        nc.sync.dma_start(
            wk[:].rearrange("p (c f) -> p c f", c=8),
            wk_d.ap().rearrange("(c p) f -> p c f", p=128),
        )
        # remaining weights/constants via gpsimd (SWDGE) so they don't
        # queue ahead of the phase-1/2 x-tile loads on the sync ring
        nc.gpsimd.dma_start(
            wv[:].rearrange("p (c f) -> p c f", c=8),
            wv_d.ap().rearrange("(c p) f -> p c f", p=128),
        )
        nc.gpsimd.dma_start(
            wq[:].rearrange("p (c f) -> p c f", c=8),
            wq_d.ap().rearrange("(c p) f -> p c f", p=128),
        )
        nc.gpsimd.dma_start(
            wo[:].rearrange("p (c e) -> p c e", c=4),
            wo_d.ap().rearrange("(c p) e -> p c e", p=64),
        )
        nc.gpsimd.dma_start(sbc[:], sbc_d.ap())
        # static contract rows 64..95 of qT/kT, replicated per head fold
        nc.gpsimd.dma_start(
            qT[64:96, :].rearrange("p (h s) -> p h s", h=HPC),
            qhot_d.ap().rearrange("p s -> p () s").broadcast_to((32, HPC, S)),
        )
        nc.gpsimd.dma_start(
            kT[64:96, :].rearrange("p (h s) -> p h s", h=HPC),
            kband_d.ap().rearrange("p s -> p () s").broadcast_to((32, HPC, S)),
        )

        # ---- phase 1: k projection (kT layout [head, f, s]) ----
        with tc.tile_pool(name="xk", bufs=2) as xkp, tc.tile_pool(
            name="psK", bufs=1, space="PSUM"
        ) as pskp:
            psK = pskp.tile([128, 4096], F32)
            for e in range(8):
                xt = xkp.tile([128, S], BF16, tag="xk")
                nc.sync.dma_start(xt[:], xk_d.ap()[e * 128 : (e + 1) * 128, :])
                for fold in range(2):
                    for sc in range(4):
                        nc.tensor.matmul(
                            psK[:, (fold * 4 + sc) * 512 : (fold * 4 + sc + 1) * 512],
                            wk[:, e * F + fold * 128 : e * F + fold * 128 + 128],
                            xt[:, sc * 512 : (sc + 1) * 512],
                            start=(e == 0),
                            stop=(e == 7),
                        )
            for fold in range(2):
                for sc in range(4):
                    src = psK[:, (fold * 4 + sc) * 512 : (fold * 4 + sc + 1) * 512]
                    h0, h1 = 2 * fold, 2 * fold + 1
                    nc.scalar.copy(
                        kT[0:64, h0 * S + sc * 512 : h0 * S + (sc + 1) * 512],
                        src[0:64, :],
                    )
                    nc.scalar.copy(
                        kT[0:64, h1 * S + sc * 512 : h1 * S + (sc + 1) * 512],
                        src[64:128, :],
                    )

        # ---- phase 2: v projection (natural layout [s, f]) ----
        with tc.tile_pool(name="xv", bufs=3) as xvp, tc.tile_pool(
            name="psV", bufs=2, space="PSUM"
        ) as psvp:
            for sc in range(4):
                pvs = [
                    psvp.tile([128, 256], F32, name=f"pv{sub}", tag=f"psV{sub}")
                    for sub in range(4)
                ]
                for e in range(8):
                    xt = xvp.tile([128, 512], BF16, tag="xv")
                    nc.sync.dma_start(
                        xt[:],
                        xv_d.ap()[e * 128 : (e + 1) * 128, sc * 512 : (sc + 1) * 512],
                    )
                    for sub in range(4):
                        nc.tensor.matmul(
                            pvs[sub][:],
                            xt[:, sub * 128 : (sub + 1) * 128],
                            wv[:, e * F : (e + 1) * F],
                            start=(e == 0),
                            stop=(e == 7),
                        )
                for sub in range(4):
                    nc.scalar.copy(
                        vv[:, sc * 1024 + sub * 256 : sc * 1024 + (sub + 1) * 256],
                        pvs[sub][:],
                    )

        # ---- phase 3: q projection + attention + output projection ----
        xqp = ctx.enter_context(tc.tile_pool(name="xq", bufs=3))
        psSp = ctx.enter_context(tc.tile_pool(name="psS", bufs=6, space="PSUM"))
        flexp = ctx.enter_context(tc.tile_pool(name="flex", bufs=2, space="PSUM"))
        expp = ctx.enter_context(tc.tile_pool(name="expS", bufs=2))
        ptp = ctx.enter_context(tc.tile_pool(name="pt", bufs=4))
        rbsp = ctx.enter_context(tc.tile_pool(name="rbs", bufs=4))
        attp = ctx.enter_context(tc.tile_pool(name="att", bufs=8))
        outp = ctx.enter_context(tc.tile_pool(name="outsb", bufs=2))

        def qproj(t):
            pqs = [
                psSp.tile([128, 512], F32, name=f"pq{fold}", tag="psS")
                for fold in range(2)
            ]
            for e in range(8):
                xt = xqp.tile([128, 512], BF16, tag="xq")
                nc.sync.dma_start(
                    xt[:],
                    xq_d.ap()[e * 128 : (e + 1) * 128, t * 512 : (t + 1) * 512],
                )
                for fold in range(2):
                    nc.tensor.matmul(
                        pqs[fold][:],
                        wq[:, e * F + fold * 128 : e * F + fold * 128 + 128],
                        xt[:],
                        start=(e == 0),
                        stop=(e == 7),
                    )
            for fold in range(2):
                h0, h1 = 2 * fold, 2 * fold + 1
                nc.scalar.copy(
                    qT[0:64, h0 * S + t * QS : h0 * S + (t + 1) * QS],
                    pqs[fold][0:64, :],
                )
                nc.scalar.copy(
                    qT[0:64, h1 * S + t * QS : h1 * S + (t + 1) * QS],
                    pqs[fold][64:128, :],
                )

        def unit(h, t, attn_t):
            npt = NP_T[t]
            lo = LO[t]
            order = PAIR_ORDER[t]
            expS = expp.tile([128, MAXP * QS], BF16, tag="expS")
            acco = psSp.tile([128, 512], F32, name="acco", tag="psS")

            def scores(i):
                j = order[i]
                c0 = lo + 2 * j
                qlo, qhi = QRANGE[t][j]
                w = qhi - qlo
                ps = psSp.tile([128, 512], F32, name="ps", tag="psS")
                nc.tensor.matmul(
                    ps[:, 0:w],
                    kT[0:96, h * S + c0 * 64 : h * S + c0 * 64 + 128],
                    qT[0:96, h * S + t * QS + qlo : h * S + t * QS + qhi],
                    start=True,
                    stop=True,
                )
                nc.scalar.activation(
                    expS[:, j * QS + qlo : j * QS + qhi], ps[:, 0:w], EXP
                )

            def sums(i):
                j = order[i]
                qlo, qhi = QRANGE[t][j]
                w = qhi - qlo
                bs = psSp.tile([128, 512], F32, name="bs", tag="psS")
                nc.tensor.matmul(
                    bs[:, 0:w],
                    sbc[:, :],
                    expS[:, j * QS + qlo : j * QS + qhi],
                    start=True,
                    stop=True,
                )
                rbs = rbsp.tile([128, 512], F32, tag="rbs")
                nc.vector.reciprocal_approx_fast(rbs[:, 0:w], bs[:, 0:w])
                pt = ptp.tile([128, 512], BF16, tag="pt")
                nc.vector.tensor_mul(
                    pt[:, 0:w], expS[:, j * QS + qlo : j * QS + qhi], rbs[:, 0:w]
                )
                return pt

            def av(i, pt):
                j = order[i]
                c0 = lo + 2 * j
                qlo, qhi = QRANGE[t][j]
                w = qhi - qlo
                cp = c0 // 2
                nc.tensor.matmul(
                    acco[0:64, qlo:qhi],
                    vv[:, cp * F + h * 64 : cp * F + h * 64 + 64],
                    pt[:, 0:w],
                    start=(i == 0),
                    stop=(i == npt - 1),
                    skip_group_check=True,
                )

            scores(0)
            if npt > 1:
                scores(1)
            pts = {}
            for i in range(npt):
                pts[i] = sums(i)
                if i + 2 < npt:
                    scores(i + 2)
                if i >= 1:
                    av(i - 1, pts.pop(i - 1))
            av(npt - 1, pts.pop(npt - 1))
            nc.scalar.copy(attn_t[:, :], acco[0:64, :])

        def outproj(t, atts):
            for sc2 in range(4):
                ob = outp.tile([128, 1024], F32, tag="outsb")
                for eh in range(2):
                    po = flexp.tile([128, 512], F32, tag="flex")
                    for h in range(HPC):
                        nc.tensor.matmul(
                            po[:],
                            atts[h][:, sc2 * 128 : sc2 * 128 + 128],
                            wo[:, h * E + eh * 512 : h * E + eh * 512 + 512],
                            start=(h == 0),
                            stop=(h == HPC - 1),
                        )
                    nc.scalar.copy(ob[:, eh * 512 : (eh + 1) * 512], po[:])
                row = (4 * t + sc2) * 128
                nc.gpsimd.dma_start(out_d.ap()[row : row + 128, :], ob[:])

        units = [(t, h) for t in range(T_SLABS) for h in range(HPC)]
        atts_by_t = {t: [] for t in range(T_SLABS)}
        for t, h in units:
            if h == 0:
                qproj(t)
            attn_t = attp.tile([64, 512], BF16, tag="att")
            unit(h, t, attn_t)
            atts_by_t[t].append(attn_t)
            if len(atts_by_t[t]) == HPC:
                outproj(t, atts_by_t[t])

    nc.compile()
    return nc


_NC_CACHE = []


def _get_nc():
    if not _NC_CACHE:
        _NC_CACHE.append(build_nc())
    return _NC_CACHE[0]


def _host_consts():
    qhot = np.zeros((32, S), np.float32)
    for s in range(S):
        qhot[s // BLK, s] = 1.0
    kband = np.zeros((32, S), np.float32)
    for k in range(S):
        c = k // BLK
        for r in range(32):
            if abs(r - c) > BAND:
                kband[r, k] = BIGNEG
    sbc = np.full((128, 128), EPS_BG, np.float32)
    for k in range(128):
        for p in range(128):
            if k // 64 == p // 64:
                sbc[k, p] = 1.0
    return qhot, kband, sbc


def make_in_maps(query, key, value, Wq, Wk, Wv, Wo):
    query = np.asarray(query, np.float32)
    key = np.asarray(key, np.float32)
    value = np.asarray(value, np.float32)
    Wq = np.asarray(Wq, np.float32)
    Wk = np.asarray(Wk, np.float32)
    Wv = np.asarray(Wv, np.float32)
    Wo = np.asarray(Wo, np.float32)

    qhot, kband, sbc = _host_consts()

    in_maps = []
    for c in range(NCORES):
        b, g = divmod(c, HPC)
        fs = slice(F * g, F * (g + 1))
        in_maps.append(
            {
                "xqT": np.ascontiguousarray(query[b].T).astype(BF16NP),
                "xkT": np.ascontiguousarray(key[b].T).astype(BF16NP),
                "xvT": np.ascontiguousarray(value[b].T).astype(BF16NP),
                "wqT": np.ascontiguousarray((Wq[fs, :] * SCALE).T).astype(BF16NP),
                "wkT": np.ascontiguousarray(Wk[fs, :].T).astype(BF16NP),
                "wvT": np.ascontiguousarray(Wv[fs, :].T).astype(BF16NP),
                "woT": np.ascontiguousarray(Wo[:, fs].T).astype(BF16NP),
                "qhot": qhot.astype(BF16NP),
                "kband": kband.astype(BF16NP),
                "sbc": sbc.astype(BF16NP),
            }
        )
    return in_maps


def kernel(query, key, value, Wq, Wk, Wv, Wo):
    nc = _get_nc()
    in_maps = make_in_maps(query, key, value, Wq, Wk, Wv, Wo)
    res = bass_utils.run_bass_kernel_spmd(nc, in_maps, core_ids=list(range(NCORES)))
    out = np.zeros((B, S, E), np.float32)
    for c in range(NCORES):
        b = c // HPC
        out[b] += res.results[c]["out"]
    return out


# revision 6
# speedup vs baseline: 1.7433x; 1.2510x over previous
"""Block-sparse (banded) attention kernel for Trainium2, 8 NeuronCores.

Sharding: data-parallel over batch (2) x tensor-parallel over heads
(16 heads -> 4 per core).  Each core computes its 4 heads' Q/K/V
projections, banded block attention (|r-c| <= 15 blocks, per-block
softmax), and a partial output projection; the host sums the 4 partial
outputs per batch element.

V2 structure: the band mask is folded into the scores matmul via 32
static contract rows (one-hot q-block indicator on the moving side x
-3e4 band-complement table on the stationary side), so masked scores
exp to exactly 0.  Per-block softmax denominators come from ONE matmul
with a block-membership (+eps) stationary whose output is already
broadcast across partitions; reciprocal runs per pair on the vector
engine.  Each pair only processes its valid contiguous q-range.

Self-contained: hardcodes all shapes; only needs the concourse tree that
the environment already puts on sys.path.
"""

import sys

for _p in ("/opt/trn_rl_repo",):
    if _p not in sys.path:
        sys.path.insert(0, _p)

from contextlib import ExitStack

import numpy as np
import ml_dtypes

import concourse.bacc as bacc
import concourse.tile as tile
from concourse import bass_utils, mybir

F32 = mybir.dt.float32
BF16 = mybir.dt.bfloat16
EXP = mybir.ActivationFunctionType.Exp
BF16NP = ml_dtypes.bfloat16

B, S, E = 2, 2048, 1024
H, HD, BLK = 16, 64, 64
NB = S // BLK  # 32 blocks
NCORES = 8
HPC = 4  # heads per core
F = HPC * HD  # 256 local features
BAND = 15
SCALE = HD ** -0.5
BIGNEG = -30000.0  # masked-score bias; exp underflows to exactly 0 in f32
EPS_BG = 1e-20  # background weight in the sum stationary: keeps denom > 0

# per r8-slab (8 query blocks, q=512) column-block ranges, even-extended
T_SLABS = 4
QS = 512  # q extent per slab
LO = []
NP_T = []
for _t in range(T_SLABS):
    lo = max(0, 8 * _t - BAND)
    hi = min(NB - 1, 8 * _t + 7 + BAND)
    if (hi - lo + 1) % 2 == 1:
        if lo > 0:
            lo -= 1
        else:
            hi += 1
    LO.append(lo)
    NP_T.append((hi - lo + 1) // 2)
MAXP = max(NP_T)  # 16 pairs

# per (slab, pair): valid contiguous local q-block range [lb, ub]
#   union of the two blocks' bands: global r in [c0-15, c0+16]
QRANGE = []  # QRANGE[t][j] = (qlo, qhi) in elements within the slab
PAIR_ORDER = []  # full-width pair first (accumulation-group opener)
for _t in range(T_SLABS):
    rng = []
    for _j in range(NP_T[_t]):
        c0 = LO[_t] + 2 * _j
        lb = max(0, c0 - BAND - 8 * _t)
        ub = min(7, c0 + BAND + 1 - 8 * _t)
        assert lb <= ub
        rng.append((lb * BLK, (ub + 1) * BLK))
    QRANGE.append(rng)
    full = [j for j, (a, b) in enumerate(rng) if b - a == QS]
    order = [full[0]] + [j for j in range(NP_T[_t]) if j != full[0]]
    PAIR_ORDER.append(order)


def build_nc():
    nc = bacc.Bacc("TRN2", target_bir_lowering=False, debug=False)

    xq_d = nc.dram_tensor("xqT", [E, S], BF16, kind="ExternalInput")
    xk_d = nc.dram_tensor("xkT", [E, S], BF16, kind="ExternalInput")
    xv_d = nc.dram_tensor("xvT", [E, S], BF16, kind="ExternalInput")
    wq_d = nc.dram_tensor("wqT", [E, F], BF16, kind="ExternalInput")
    wk_d = nc.dram_tensor("wkT", [E, F], BF16, kind="ExternalInput")
    wv_d = nc.dram_tensor("wvT", [E, F], BF16, kind="ExternalInput")
    wo_d = nc.dram_tensor("woT", [F, E], BF16, kind="ExternalInput")
    qhot_d = nc.dram_tensor("qhot", [32, S], BF16, kind="ExternalInput")
    kband_d = nc.dram_tensor("kband", [32, S], BF16, kind="ExternalInput")
    sbc_d = nc.dram_tensor("sbc", [128, 128], BF16, kind="ExternalInput")
    out_d = nc.dram_tensor("out", [S, E], F32, kind="ExternalOutput")

    with tile.TileContext(nc) as tc, ExitStack() as ctx, nc.allow_low_precision(
        reason="bf16 pipeline; fp32 PSUM accumulate throughout"
    ):
        pers = ctx.enter_context(tc.tile_pool(name="pers", bufs=1))
        qT = pers.tile([96, HPC * S], BF16, tag="qT")
        kT = pers.tile([96, HPC * S], BF16, tag="kT")
        vv = pers.tile([128, 16 * F], BF16, tag="vv")
        wq = pers.tile([128, 8 * F], BF16, tag="wq")
        wk = pers.tile([128, 8 * F], BF16, tag="wk")
        wv = pers.tile([128, 8 * F], BF16, tag="wv")
        wo = pers.tile([64, 4 * E], BF16, tag="wo")
        sbc = pers.tile([128, 128], BF16, tag="sbc")

        # k-projection weights first: phase 1 is on the critical path
        nc.sync.dma_start(
            wk[:].rearrange("p (c f) -> p c f", c=8),
            wk_d.ap().rearrange("(c p) f -> p c f", p=128),
        )
        # remaining weights/constants via gpsimd (SWDGE) so they don't
        # queue ahead of the phase-1/2 x-tile loads on the sync ring
        nc.gpsimd.dma_start(
            wv[:].rearrange("p (c f) -> p c f", c=8),
            wv_d.ap().rearrange("(c p) f -> p c f", p=128),
        )
        nc.gpsimd.dma_start(
            wq[:].rearrange("p (c f) -> p c f", c=8),
            wq_d.ap().rearrange("(c p) f -> p c f", p=128),
        )
        nc.gpsimd.dma_start(
            wo[:].rearrange("p (c e) -> p c e", c=4),
            wo_d.ap().rearrange("(c p) e -> p c e", p=64),
        )
        nc.gpsimd.dma_start(sbc[:], sbc_d.ap())
        # static contract rows 64..95 of qT/kT, replicated per head fold
        nc.gpsimd.dma_start(
            qT[64:96, :].rearrange("p (h s) -> p h s", h=HPC),
            qhot_d.ap().rearrange("p s -> p () s").broadcast_to((32, HPC, S)),
        )
        nc.gpsimd.dma_start(
            kT[64:96, :].rearrange("p (h s) -> p h s", h=HPC),
            kband_d.ap().rearrange("p s -> p () s").broadcast_to((32, HPC, S)),
        )

        # ---- phase 1: k projection (kT layout [head, f, s]) ----
        with tc.tile_pool(name="xk", bufs=2) as xkp, tc.tile_pool(
            name="psK", bufs=1, space="PSUM"
        ) as pskp:
            psK = pskp.tile([128, 4096], F32)
            for e in range(8):
                xt = xkp.tile([128, S], BF16, tag="xk")
                nc.sync.dma_start(xt[:], xk_d.ap()[e * 128 : (e + 1) * 128, :])
                for fold in range(2):
                    for sc in range(4):
                        nc.tensor.matmul(
                            psK[:, (fold * 4 + sc) * 512 : (fold * 4 + sc + 1) * 512],
                            wk[:, e * F + fold * 128 : e * F + fold * 128 + 128],
                            xt[:, sc * 512 : (sc + 1) * 512],
                            start=(e == 0),
                            stop=(e == 7),
                        )
            for fold in range(2):
                for sc in range(4):
                    src = psK[:, (fold * 4 + sc) * 512 : (fold * 4 + sc + 1) * 512]
                    h0, h1 = 2 * fold, 2 * fold + 1
                    nc.scalar.copy(
                        kT[0:64, h0 * S + sc * 512 : h0 * S + (sc + 1) * 512],
                        src[0:64, :],
                    )
                    nc.scalar.copy(
                        kT[0:64, h1 * S + sc * 512 : h1 * S + (sc + 1) * 512],
                        src[64:128, :],
                    )

        # ---- phase 2: v projection (natural layout [s, f]) ----
        with tc.tile_pool(name="xv", bufs=3) as xvp, tc.tile_pool(
            name="psV", bufs=2, space="PSUM"
        ) as psvp:
            for sc in range(4):
                pvs = [
                    psvp.tile([128, 256], F32, name=f"pv{sub}", tag=f"psV{sub}")
                    for sub in range(4)
                ]
                for e in range(8):
                    xt = xvp.tile([128, 512], BF16, tag="xv")
                    nc.sync.dma_start(
                        xt[:],
                        xv_d.ap()[e * 128 : (e + 1) * 128, sc * 512 : (sc + 1) * 512],
                    )
                    for sub in range(4):
                        nc.tensor.matmul(
                            pvs[sub][:],
                            xt[:, sub * 128 : (sub + 1) * 128],
                            wv[:, e * F : (e + 1) * F],
                            start=(e == 0),
                            stop=(e == 7),
                        )
                for sub in range(4):
                    nc.scalar.copy(
                        vv[:, sc * 1024 + sub * 256 : sc * 1024 + (sub + 1) * 256],
                        pvs[sub][:],
                    )

        # ---- phase 3: q projection + attention + output projection ----
        xqp = ctx.enter_context(tc.tile_pool(name="xq", bufs=3))
        psSp = ctx.enter_context(tc.tile_pool(name="psS", bufs=6, space="PSUM"))
        flexp = ctx.enter_context(tc.tile_pool(name="flex", bufs=2, space="PSUM"))
        expp = ctx.enter_context(tc.tile_pool(name="expS", bufs=2))
        ptp = ctx.enter_context(tc.tile_pool(name="pt", bufs=4))
        rbsp = ctx.enter_context(tc.tile_pool(name="rbs", bufs=4))
        attp = ctx.enter_context(tc.tile_pool(name="att", bufs=8))
        outp = ctx.enter_context(tc.tile_pool(name="outsb", bufs=2))

        def qproj(t):
            pqs = [
                psSp.tile([128, 512], F32, name=f"pq{fold}", tag="psS")
                for fold in range(2)
            ]
            for e in range(8):
                xt = xqp.tile([128, 512], BF16, tag="xq")
                nc.sync.dma_start(
                    xt[:],
                    xq_d.ap()[e * 128 : (e + 1) * 128, t * 512 : (t + 1) * 512],
                )
                for fold in range(2):
                    nc.tensor.matmul(
                        pqs[fold][:],
                        wq[:, e * F + fold * 128 : e * F + fold * 128 + 128],
                        xt[:],
                        start=(e == 0),
                        stop=(e == 7),
                    )
            for fold in range(2):
                h0, h1 = 2 * fold, 2 * fold + 1
                nc.scalar.copy(
                    qT[0:64, h0 * S + t * QS : h0 * S + (t + 1) * QS],
                    pqs[fold][0:64, :],
                )
                nc.scalar.copy(
                    qT[0:64, h1 * S + t * QS : h1 * S + (t + 1) * QS],
                    pqs[fold][64:128, :],
                )

        def unit(h, t, attn_t):
            npt = NP_T[t]
            lo = LO[t]
            order = PAIR_ORDER[t]
            expS = expp.tile([128, MAXP * QS], BF16, tag="expS")
            acco = psSp.tile([128, 512], F32, name="acco", tag="psS")

            def scores(i):
                j = order[i]
                c0 = lo + 2 * j
                qlo, qhi = QRANGE[t][j]
                w = qhi - qlo
                ps = psSp.tile([128, 512], F32, name="ps", tag="psS")
                nc.tensor.matmul(
                    ps[:, 0:w],
                    kT[0:96, h * S + c0 * 64 : h * S + c0 * 64 + 128],
                    qT[0:96, h * S + t * QS + qlo : h * S + t * QS + qhi],
                    start=True,
                    stop=True,
                )
                nc.scalar.activation(
                    expS[:, j * QS + qlo : j * QS + qhi], ps[:, 0:w], EXP
                )

            def sums(i):
                j = order[i]
                qlo, qhi = QRANGE[t][j]
                w = qhi - qlo
                bs = psSp.tile([128, 512], F32, name="bs", tag="psS")
                nc.tensor.matmul(
                    bs[:, 0:w],
                    sbc[:, :],
                    expS[:, j * QS + qlo : j * QS + qhi],
                    start=True,
                    stop=True,
                )
                rbs = rbsp.tile([128, 512], F32, tag="rbs")
                nc.vector.reciprocal_approx_fast(rbs[:, 0:w], bs[:, 0:w])
                pt = ptp.tile([128, 512], BF16, tag="pt")
                # alternate the normalize multiply between DVE and the idle
                # Pool engine: DVE alone can't keep up with the PE pair rate
                eng = nc.vector if i % 2 == 0 else nc.gpsimd
                eng.tensor_mul(
                    pt[:, 0:w], expS[:, j * QS + qlo : j * QS + qhi], rbs[:, 0:w]
                )
                return pt

            def av(i, pt):
                j = order[i]
                c0 = lo + 2 * j
                qlo, qhi = QRANGE[t][j]
                w = qhi - qlo
                cp = c0 // 2
                nc.tensor.matmul(
                    acco[0:64, qlo:qhi],
                    vv[:, cp * F + h * 64 : cp * F + h * 64 + 64],
                    pt[:, 0:w],
                    start=(i == 0),
                    stop=(i == npt - 1),
                    skip_group_check=True,
                )

            scores(0)
            if npt > 1:
                scores(1)
            pts = {}
            for i in range(npt):
                pts[i] = sums(i)
                if i + 2 < npt:
                    scores(i + 2)
                if i >= 2:
                    av(i - 2, pts.pop(i - 2))
            av(npt - 2, pts.pop(npt - 2))
            av(npt - 1, pts.pop(npt - 1))
            nc.scalar.copy(attn_t[:, :], acco[0:64, :])

        def outproj(t, atts):
            for sc2 in range(4):
                ob = outp.tile([128, 1024], F32, tag="outsb")
                for eh in range(2):
                    po = flexp.tile([128, 512], F32, tag="flex")
                    for h in range(HPC):
                        nc.tensor.matmul(
                            po[:],
                            atts[h][:, sc2 * 128 : sc2 * 128 + 128],
                            wo[:, h * E + eh * 512 : h * E + eh * 512 + 512],
                            start=(h == 0),
                            stop=(h == HPC - 1),
                        )
                    nc.scalar.copy(ob[:, eh * 512 : (eh + 1) * 512], po[:])
                row = (4 * t + sc2) * 128
                nc.gpsimd.dma_start(out_d.ap()[row : row + 128, :], ob[:])

        units = [(t, h) for t in range(T_SLABS) for h in range(HPC)]
        atts_by_t = {t: [] for t in range(T_SLABS)}
        for t, h in units:
            if h == 0:
                qproj(t)
            attn_t = attp.tile([64, 512], BF16, tag="att")
            unit(h, t, attn_t)
            atts_by_t[t].append(attn_t)
            if len(atts_by_t[t]) == HPC:
                outproj(t, atts_by_t[t])

    nc.compile()
    return nc


_NC_CACHE = []


def _get_nc():
    if not _NC_CACHE:
        _NC_CACHE.append(build_nc())
    return _NC_CACHE[0]


def _host_consts():
    qhot = np.zeros((32, S), np.float32)
    for s in range(S):
        qhot[s // BLK, s] = 1.0
    kband = np.zeros((32, S), np.float32)
    for k in range(S):
        c = k // BLK
        for r in range(32):
            if abs(r - c) > BAND:
                kband[r, k] = BIGNEG
    sbc = np.full((128, 128), EPS_BG, np.float32)
    for k in range(128):
        for p in range(128):
            if k // 64 == p // 64:
                sbc[k, p] = 1.0
    return qhot, kband, sbc


def make_in_maps(query, key, value, Wq, Wk, Wv, Wo):
    query = np.asarray(query, np.float32)
    key = np.asarray(key, np.float32)
    value = np.asarray(value, np.float32)
    Wq = np.asarray(Wq, np.float32)
    Wk = np.asarray(Wk, np.float32)
    Wv = np.asarray(Wv, np.float32)
    Wo = np.asarray(Wo, np.float32)

    qhot, kband, sbc = _host_consts()

    in_maps = []
    for c in range(NCORES):
        b, g = divmod(c, HPC)
        fs = slice(F * g, F * (g + 1))
        in_maps.append(
            {
                "xqT": np.ascontiguousarray(query[b].T).astype(BF16NP),
                "xkT": np.ascontiguousarray(key[b].T).astype(BF16NP),
                "xvT": np.ascontiguousarray(value[b].T).astype(BF16NP),
                "wqT": np.ascontiguousarray((Wq[fs, :] * SCALE).T).astype(BF16NP),
                "wkT": np.ascontiguousarray(Wk[fs, :].T).astype(BF16NP),
                "wvT": np.ascontiguousarray(Wv[fs, :].T).astype(BF16NP),
                "woT": np.ascontiguousarray(Wo[:, fs].T).astype(BF16NP),
                "qhot": qhot.astype(BF16NP),
                "kband": kband.astype(BF16NP),
                "sbc": sbc.astype(BF16NP),
            }
        )
    return in_maps


def kernel(query, key, value, Wq, Wk, Wv, Wo):
    nc = _get_nc()
    in_maps = make_in_maps(query, key, value, Wq, Wk, Wv, Wo)
    res = bass_utils.run_bass_kernel_spmd(nc, in_maps, core_ids=list(range(NCORES)))
    out = np.zeros((B, S, E), np.float32)
    for c in range(NCORES):
        b = c // HPC
        out[b] += res.results[c]["out"]
    return out


# revision 16
# speedup vs baseline: 1.9133x; 1.0975x over previous
"""Block-sparse (banded) attention kernel for Trainium2, 8 NeuronCores.

Sharding: data-parallel over batch (2) x tensor-parallel over heads
(16 heads -> 4 per core).  Each core computes its 4 heads' Q/K/V
projections, banded block attention (|r-c| <= 15 blocks, per-block
softmax), and a partial output projection; the host sums the 4 partial
outputs per batch element.

V2 structure: the band mask is folded into the scores matmul via 32
static contract rows (one-hot q-block indicator on the moving side x
-3e4 band-complement table on the stationary side), so masked scores
exp to exactly 0.  Per-block softmax denominators come from ONE matmul
with a block-membership (+eps) stationary whose output is already
broadcast across partitions; reciprocal runs per pair on the vector
engine.  Each pair only processes its valid contiguous q-range.

Self-contained: hardcodes all shapes; only needs the concourse tree that
the environment already puts on sys.path.
"""

import sys

for _p in ("/opt/trn_rl_repo",):
    if _p not in sys.path:
        sys.path.insert(0, _p)

from contextlib import ExitStack

import numpy as np
import ml_dtypes

import concourse.bacc as bacc
import concourse.tile as tile
from concourse import bass_utils, mybir

F32 = mybir.dt.float32
BF16 = mybir.dt.bfloat16
EXP = mybir.ActivationFunctionType.Exp
BF16NP = ml_dtypes.bfloat16

B, S, E = 2, 2048, 1024
H, HD, BLK = 16, 64, 64
NB = S // BLK  # 32 blocks
NCORES = 8
HPC = 4  # heads per core
F = HPC * HD  # 256 local features
BAND = 15
SCALE = HD ** -0.5
BIGNEG = -30000.0  # masked-score bias; exp underflows to exactly 0 in f32
EPS_BG = 1e-20  # background weight in the sum stationary: keeps denom > 0

# per r8-slab (8 query blocks, q=512) column-block ranges, even-extended
T_SLABS = 4
QS = 512  # q extent per slab
LO = []
NP_T = []
for _t in range(T_SLABS):
    lo = max(0, 8 * _t - BAND)
    hi = min(NB - 1, 8 * _t + 7 + BAND)
    if (hi - lo + 1) % 2 == 1:
        if lo > 0:
            lo -= 1
        else:
            hi += 1
    LO.append(lo)
    NP_T.append((hi - lo + 1) // 2)
MAXP = max(NP_T)  # 16 pairs

# per (slab, pair): valid contiguous local q-block range [lb, ub]
#   union of the two blocks' bands: global r in [c0-15, c0+16]
QRANGE = []  # QRANGE[t][j] = (qlo, qhi) in elements within the slab
PAIR_ORDER = []  # full-width pair first (accumulation-group opener)
for _t in range(T_SLABS):
    rng = []
    for _j in range(NP_T[_t]):
        c0 = LO[_t] + 2 * _j
        lb = max(0, c0 - BAND - 8 * _t)
        ub = min(7, c0 + BAND + 1 - 8 * _t)
        assert lb <= ub
        rng.append((lb * BLK, (ub + 1) * BLK))
    QRANGE.append(rng)
    full = [j for j, (a, b) in enumerate(rng) if b - a == QS]
    order = [full[0]] + [j for j in range(NP_T[_t]) if j != full[0]]
    PAIR_ORDER.append(order)


def build_nc():
    nc = bacc.Bacc("TRN2", target_bir_lowering=False, debug=False)

    xq_d = nc.dram_tensor("xqT", [E, S], BF16, kind="ExternalInput")
    xk_d = nc.dram_tensor("xkT", [E, S], BF16, kind="ExternalInput")
    xv_d = nc.dram_tensor("xvT", [E, S], BF16, kind="ExternalInput")
    wq_d = nc.dram_tensor("wqT", [E, F], BF16, kind="ExternalInput")
    wk_d = nc.dram_tensor("wkT", [E, F], BF16, kind="ExternalInput")
    wv_d = nc.dram_tensor("wvT", [E, F], BF16, kind="ExternalInput")
    wo_d = nc.dram_tensor("woT", [F, E], BF16, kind="ExternalInput")
    qhot_d = nc.dram_tensor("qhot", [32, S], BF16, kind="ExternalInput")
    kband_d = nc.dram_tensor("kband", [32, S], BF16, kind="ExternalInput")
    sbc_d = nc.dram_tensor("sbc", [128, 128], BF16, kind="ExternalInput")
    out_d = nc.dram_tensor("out", [S, E], BF16, kind="ExternalOutput")

    with tile.TileContext(nc) as tc, ExitStack() as ctx, nc.allow_low_precision(
        reason="bf16 pipeline; fp32 PSUM accumulate throughout"
    ):
        pers = ctx.enter_context(tc.tile_pool(name="pers", bufs=1))
        qT = pers.tile([96, HPC * S], BF16, tag="qT")
        kT = pers.tile([96, HPC * S], BF16, tag="kT")
        vv = pers.tile([128, 16 * F], BF16, tag="vv")
        wq = pers.tile([128, 8 * F], BF16, tag="wq")
        wk = pers.tile([128, 8 * F], BF16, tag="wk")
        wv = pers.tile([128, 8 * F], BF16, tag="wv")
        wo = pers.tile([64, 4 * E], BF16, tag="wo")
        sbc = pers.tile([128, 128], BF16, tag="sbc")
        xvbig = pers.tile([128, 8 * 2048], BF16, tag="xvbig")

        # k-projection weights first: phase 1 is on the critical path
        nc.sync.dma_start(
            wk[:].rearrange("p (c f) -> p c f", c=8),
            wk_d.ap().rearrange("(c p) f -> p c f", p=128),
        )
        # remaining weights/constants via gpsimd (SWDGE) so they don't
        # queue ahead of the phase-1/2 x-tile loads on the sync ring
        nc.gpsimd.dma_start(
            wv[:].rearrange("p (c f) -> p c f", c=8),
            wv_d.ap().rearrange("(c p) f -> p c f", p=128),
        )
        nc.gpsimd.dma_start(
            wq[:].rearrange("p (c f) -> p c f", c=8),
            wq_d.ap().rearrange("(c p) f -> p c f", p=128),
        )
        nc.gpsimd.dma_start(
            wo[:].rearrange("p (c e) -> p c e", c=4),
            wo_d.ap().rearrange("(c p) e -> p c e", p=64),
        )
        nc.gpsimd.dma_start(sbc[:], sbc_d.ap())
        # static contract rows 64..95 of qT/kT, replicated per head fold
        nc.gpsimd.dma_start(
            qT[64:96, :].rearrange("p (h s) -> p h s", h=HPC),
            qhot_d.ap().rearrange("p s -> p () s").broadcast_to((32, HPC, S)),
        )
        nc.gpsimd.dma_start(
            kT[64:96, :].rearrange("p (h s) -> p h s", h=HPC),
            kband_d.ap().rearrange("p s -> p () s").broadcast_to((32, HPC, S)),
        )

        # ---- phase 1: k projection (kT layout [head, f, s]) ----
        with tc.tile_pool(name="xk", bufs=2) as xkp, tc.tile_pool(
            name="psK", bufs=1, space="PSUM"
        ) as pskp:
            psK = pskp.tile([128, 4096], F32)
            for e in range(8):
                xt = xkp.tile([128, S], BF16, tag="xk")
                nc.sync.dma_start(xt[:], xk_d.ap()[e * 128 : (e + 1) * 128, :])
                if e == 0:
                    # phase-2 x preload: two grouped loads overlap phase 1
                    nc.sync.dma_start(
                        xvbig[:, 0 : 4 * 2048].rearrange("p (c s) -> p c s", c=4),
                        xv_d.ap().rearrange("(c p) s -> p c s", p=128)[:, 0:4, :],
                    )
                    nc.sync.dma_start(
                        xvbig[:, 4 * 2048 :].rearrange("p (c s) -> p c s", c=4),
                        xv_d.ap().rearrange("(c p) s -> p c s", p=128)[:, 4:8, :],
                    )
                for fold in range(2):
                    for sc in range(4):
                        nc.tensor.matmul(
                            psK[:, (fold * 4 + sc) * 512 : (fold * 4 + sc + 1) * 512],
                            wk[:, e * F + fold * 128 : e * F + fold * 128 + 128],
                            xt[:, sc * 512 : (sc + 1) * 512],
                            start=(e == 0),
                            stop=(e == 7),
                        )
            for fold in range(2):
                for sc in range(4):
                    src = psK[:, (fold * 4 + sc) * 512 : (fold * 4 + sc + 1) * 512]
                    h0, h1 = 2 * fold, 2 * fold + 1
                    nc.scalar.copy(
                        kT[0:64, h0 * S + sc * 512 : h0 * S + (sc + 1) * 512],
                        src[0:64, :],
                    )
                    nc.scalar.copy(
                        kT[0:64, h1 * S + sc * 512 : h1 * S + (sc + 1) * 512],
                        src[64:128, :],
                    )

        # ---- phase 2: v projection (natural layout [s, f]) ----
        with tc.tile_pool(name="psV", bufs=2, space="PSUM") as psvp:
            for sc in range(4):
                pvs = [
                    psvp.tile([128, 256], F32, name=f"pv{sub}", tag=f"psV{sub}")
                    for sub in range(4)
                ]
                for e in range(8):
                    for sub in range(4):
                        nc.tensor.matmul(
                            pvs[sub][:],
                            xvbig[:, e * 2048 + sc * 512 + sub * 128 :
                                  e * 2048 + sc * 512 + (sub + 1) * 128],
                            wv[:, e * F : (e + 1) * F],
                            start=(e == 0),
                            stop=(e == 7),
                        )
                for sub in range(4):
                    nc.scalar.copy(
                        vv[:, sc * 1024 + sub * 256 : sc * 1024 + (sub + 1) * 256],
                        pvs[sub][:],
                    )

        # ---- phase 3: q projection + attention + output projection ----
        xqp = ctx.enter_context(tc.tile_pool(name="xq", bufs=2))
        psSp = ctx.enter_context(tc.tile_pool(name="psS", bufs=6, space="PSUM"))
        flexp = ctx.enter_context(tc.tile_pool(name="flex", bufs=2, space="PSUM"))
        expp = ctx.enter_context(tc.tile_pool(name="expS", bufs=2))
        ptp = ctx.enter_context(tc.tile_pool(name="pt", bufs=4))
        rbsp = ctx.enter_context(tc.tile_pool(name="rbs", bufs=4))
        attp = ctx.enter_context(tc.tile_pool(name="att", bufs=8))
        outp = ctx.enter_context(tc.tile_pool(name="outsb", bufs=2))

        xq_tiles = {}

        def qproj_load(t):
            # one grouped load for the whole slab's x columns
            xt = xqp.tile([128, 8 * 512], BF16, tag="xq")
            nc.sync.dma_start(
                xt[:].rearrange("p (c s) -> p c s", c=8),
                xq_d.ap().rearrange("(c p) s -> p c s", p=128)[
                    :, :, t * 512 : (t + 1) * 512
                ],
            )
            xq_tiles[t] = xt

        def qproj_mm(t):
            xt = xq_tiles.pop(t)
            pqs = [
                flexp.tile([128, 512], F32, name=f"pq{fold}", tag="flex")
                for fold in range(2)
            ]
            for e in range(8):
                for fold in range(2):
                    nc.tensor.matmul(
                        pqs[fold][:],
                        wq[:, e * F + fold * 128 : e * F + fold * 128 + 128],
                        xt[:, e * 512 : (e + 1) * 512],
                        start=(e == 0),
                        stop=(e == 7),
                    )
            for fold in range(2):
                h0, h1 = 2 * fold, 2 * fold + 1
                nc.scalar.copy(
                    qT[0:64, h0 * S + t * QS : h0 * S + (t + 1) * QS],
                    pqs[fold][0:64, :],
                )
                nc.scalar.copy(
                    qT[0:64, h1 * S + t * QS : h1 * S + (t + 1) * QS],
                    pqs[fold][64:128, :],
                )

        def outproj(t, atts):
            for sc2 in range(4):
                ob = outp.tile([128, 1024], BF16, tag="outsb")
                for eh in range(2):
                    po = flexp.tile([128, 512], F32, tag="flex")
                    for h in range(HPC):
                        nc.tensor.matmul(
                            po[:],
                            atts[h][:, sc2 * 128 : sc2 * 128 + 128],
                            wo[:, h * E + eh * 512 : h * E + eh * 512 + 512],
                            start=(h == 0),
                            stop=(h == HPC - 1),
                        )
                    nc.scalar.copy(ob[:, eh * 512 : (eh + 1) * 512], po[:])
                row = (4 * t + sc2) * 128
                nc.gpsimd.dma_start(out_d.ap()[row : row + 128, :], ob[:])

        # flat software pipeline over every (slab, head, pair) slot
        SLOTS = [
            (t, h, i)
            for t in range(T_SLABS)
            for h in range(HPC)
            for i in range(NP_T[t])
        ]
        N = len(SLOTS)
        ctx = {}
        atts_by_t = {t: [] for t in range(T_SLABS)}

        def ensure_ctx(t, h):
            if (t, h) in ctx:
                return
            if h == 0 and t == 0:
                qproj_mm(0)
            ctx[(t, h)] = {
                "expS": expp.tile([128, MAXP * QS], BF16, name="expS", tag="expS"),
                "acco": psSp.tile([128, 512], F32, name="acco", tag="psS"),
                "attn": attp.tile([64, 512], BF16, name="attn", tag="att"),
                "pts": {},
            }
            if t + 1 < T_SLABS:
                if h == HPC - 2:
                    # start next slab's x transfer one unit before its matmuls
                    qproj_load(t + 1)
                elif h == HPC - 1:
                    qproj_mm(t + 1)

        def do_scores(g):
            t, h, i = SLOTS[g]
            ensure_ctx(t, h)
            c = ctx[(t, h)]
            j = PAIR_ORDER[t][i]
            c0 = LO[t] + 2 * j
            qlo, qhi = QRANGE[t][j]
            w = qhi - qlo
            ps = psSp.tile([128, 512], F32, name="ps", tag="psS")
            nc.tensor.matmul(
                ps[:, 0:w],
                kT[0:96, h * S + c0 * 64 : h * S + c0 * 64 + 128],
                qT[0:96, h * S + t * QS + qlo : h * S + t * QS + qhi],
                start=True,
                stop=True,
            )
            nc.scalar.activation(
                c["expS"][:, j * QS + qlo : j * QS + qhi], ps[:, 0:w], EXP
            )

        def do_sums(g):
            t, h, i = SLOTS[g]
            c = ctx[(t, h)]
            j = PAIR_ORDER[t][i]
            qlo, qhi = QRANGE[t][j]
            w = qhi - qlo
            bs = psSp.tile([128, 512], F32, name="bs", tag="psS")
            nc.tensor.matmul(
                bs[:, 0:w],
                sbc[:, :],
                c["expS"][:, j * QS + qlo : j * QS + qhi],
                start=True,
                stop=True,
            )
            rbs = rbsp.tile([128, 512], F32, tag="rbs")
            nc.vector.reciprocal_approx_fast(rbs[:, 0:w], bs[:, 0:w])
            pt = ptp.tile([128, 512], BF16, tag="pt")
            # alternate the normalize multiply between DVE and the idle Pool
            # engine (DVE alone can't keep the PE pair rate); keep unit-tail
            # multiplies on the faster DVE so drains stay short
            eng = nc.gpsimd if (i % 2 == 1 and i < NP_T[t] - 3) else nc.vector
            eng.tensor_mul(
                pt[:, 0:w], c["expS"][:, j * QS + qlo : j * QS + qhi], rbs[:, 0:w]
            )
            c["pts"][i] = pt

        def do_av(g):
            t, h, i = SLOTS[g]
            c = ctx[(t, h)]
            j = PAIR_ORDER[t][i]
            c0 = LO[t] + 2 * j
            qlo, qhi = QRANGE[t][j]
            w = qhi - qlo
            cp = c0 // 2
            pt = c["pts"].pop(i)
            nc.tensor.matmul(
                c["acco"][0:64, qlo:qhi],
                vv[:, cp * F + h * 64 : cp * F + h * 64 + 64],
                pt[:, 0:w],
                start=(i == 0),
                stop=(i == NP_T[t] - 1),
                skip_group_check=True,
            )
            if i == NP_T[t] - 1:
                nc.scalar.copy(c["attn"][:, :], c["acco"][0:64, :])
                atts_by_t[t].append(c["attn"])
                if len(atts_by_t[t]) == HPC:
                    outproj(t, atts_by_t[t])

        qproj_load(0)  # transfer overlaps phase-2 compute
        do_scores(0)
        do_scores(1)
        for g in range(N):
            do_sums(g)
            if g + 2 < N:
                do_scores(g + 2)
            if g >= 2:
                do_av(g - 2)
        do_av(N - 2)
        do_av(N - 1)

    nc.compile()
    return nc


_NC_CACHE = []


def _get_nc():
    if not _NC_CACHE:
        _NC_CACHE.append(build_nc())
    return _NC_CACHE[0]


def _host_consts():
    qhot = np.zeros((32, S), np.float32)
    for s in range(S):
        qhot[s // BLK, s] = 1.0
    kband = np.zeros((32, S), np.float32)
    for k in range(S):
        c = k // BLK
        for r in range(32):
            if abs(r - c) > BAND:
                kband[r, k] = BIGNEG
    sbc = np.full((128, 128), EPS_BG, np.float32)
    for k in range(128):
        for p in range(128):
            if k // 64 == p // 64:
                sbc[k, p] = 1.0
    return qhot, kband, sbc


def make_in_maps(query, key, value, Wq, Wk, Wv, Wo):
    query = np.asarray(query, np.float32)
    key = np.asarray(key, np.float32)
    value = np.asarray(value, np.float32)
    Wq = np.asarray(Wq, np.float32)
    Wk = np.asarray(Wk, np.float32)
    Wv = np.asarray(Wv, np.float32)
    Wo = np.asarray(Wo, np.float32)

    qhot, kband, sbc = _host_consts()

    in_maps = []
    for c in range(NCORES):
        b, g = divmod(c, HPC)
        fs = slice(F * g, F * (g + 1))
        in_maps.append(
            {
                "xqT": np.ascontiguousarray(query[b].T).astype(BF16NP),
                "xkT": np.ascontiguousarray(key[b].T).astype(BF16NP),
                "xvT": np.ascontiguousarray(value[b].T).astype(BF16NP),
                "wqT": np.ascontiguousarray((Wq[fs, :] * SCALE).T).astype(BF16NP),
                "wkT": np.ascontiguousarray(Wk[fs, :].T).astype(BF16NP),
                "wvT": np.ascontiguousarray(Wv[fs, :].T).astype(BF16NP),
                "woT": np.ascontiguousarray(Wo[:, fs].T).astype(BF16NP),
                "qhot": qhot.astype(BF16NP),
                "kband": kband.astype(BF16NP),
                "sbc": sbc.astype(BF16NP),
            }
        )
    return in_maps


def kernel(query, key, value, Wq, Wk, Wv, Wo):
    nc = _get_nc()
    in_maps = make_in_maps(query, key, value, Wq, Wk, Wv, Wo)
    res = bass_utils.run_bass_kernel_spmd(nc, in_maps, core_ids=list(range(NCORES)))
    out = np.zeros((B, S, E), np.float32)
    for c in range(NCORES):
        b = c // HPC
        out[b] += res.results[c]["out"]
    return out
